# revision 4
# baseline (speedup 1.0000x reference)
"""GQA attention forward (dense_transformer), 8-core tensor-parallel Bass kernel.

Problem (hardcoded): B=2, S=1024, H=4096, n_kv=8, G=8 (heads/kv), D=64, f32 io.
Sharding: core m owns kv-group m (8 q-heads + 1 kv-head), computes its slice
attnT_m = [512, 2048] of the attention output (feature-on-partition transposed
layout), AllGathers attnT (bf16, chunked per 512-token block for overlap),
then computes output columns y[:, m*512:(m+1)*512] = attn @ wd[m*512:(m+1)*512, :].T,
emitted transposed (yT); the host un-transposes and concatenates.

v2 schedule (single program, engines kept busy):
 - hs streamed in 4-htile chunks (both batches) so the PE starts ~5us in.
 - attention(b0) interleaved at granule level with q-proj(b1) so the
   scores->exp->PV chain never idles the PE (p-state stays high).
 - causal-trimmed scores/exp/PV streams (only s >= t columns) -- no mask
   memsets needed, 25% fewer PE columns.
 - AllGather fired per (batch, 512-token block): 4 small collectives that
   overlap attention compute; dense streams gathered activations in chunks.
 - softmax denominators: ones-row in V gives sums; one batched gpsimd
   partition_broadcast per 4 heads for the 1/sum scale.

All matmuls bf16 (rel-err budget 2e-2); softmax skips max-subtraction
(logits bounded ~|7|); probs stay unnormalized through PV; output scaled by
1/sum. The BIR shim drops Ldweights whose stationary matches the previous
load, so same-stationary matmuls are emitted back to back.
"""

import sys

import numpy as np

for _p in ("/opt/trn_rl_repo",):
    if _p not in sys.path:
        sys.path.insert(0, _p)

import ml_dtypes

B, S, H = 2, 1024, 4096
NKV, G, D = 8, 8, 64
NC = 8
BS = B * S          # 2048 flattened tokens
EL = G * D          # 512 local attn features per core
HT = H // 128       # 32 h-tiles
SBK = 512           # s-block everywhere
NBLK = S // SBK     # 2 s-blocks per batch
NCH = 8             # hs chunks per batch (4 h-tiles each)
CHA = HT // NCH     # h-tiles per chunk = 4
INV = 0.125         # 1/sqrt(D)

_CACHE = {}


def _fix_bir_for_old_walrus(bir_json):
    """Adapt newer-concourse BIR to the container's older walrus:
    - register allocations need num_physical_regs set
    - only one sem-wait per instruction: hoist extras onto EventSemaphore nops
    - drop Ldweights identical to the previous one (stationary persists in
      the PE array; verified bit-exact on hardware)
    """
    import json

    bir = json.loads(bir_json)
    ndrop = 0
    for f in bir["functions"]:
        for a in f["allocations"]:
            if a.get("Skind") == "register" and not a.get("num_physical_regs"):
                a["num_physical_regs"] = 1
        for blk in f["blocks"]:
            newins = []
            last_ldw = None
            for ins in blk["instructions"]:
                si = ins.get("sync_info") or {}
                waits = si.get("on_wait") or []
                if len(waits) > 1:
                    for j, w in enumerate(waits[:-1]):
                        newins.append(
                            {
                                "engine": ins["engine"],
                                "ins": [],
                                "outs": [],
                                "name": f"{ins['name']}_w{j}",
                                "opcode": "EventSemaphore",
                                "sync_info": {"on_update": [], "on_wait": [w]},
                                "debug": ins.get("debug"),
                            }
                        )
                    si["on_wait"] = [waits[-1]]
                op = ins.get("opcode")
                if op == "Ldweights":
                    key = json.dumps(ins["ins"], sort_keys=True)
                    sync = ins.get("sync_info") or {}
                    if (
                        key == last_ldw
                        and not sync.get("on_wait")
                        and not sync.get("on_update")
                    ):
                        ndrop += 1
                        continue
                    last_ldw = key
                elif op != "Matmult":
                    last_ldw = None
                newins.append(ins)
            blk["instructions"] = newins
    return json.dumps(bir).encode()


def _install_compiler_shim():
    if _CACHE.get("shim"):
        return
    import concourse.bass_utils as bu
    import concourse.bass2jax as b2j

    orig = getattr(bu.compile_bir_kernel, "__wrapped__", bu.compile_bir_kernel)

    def patched(bir_json, tmpdir, neff_name="file.neff"):
        return orig(_fix_bir_for_old_walrus(bir_json), tmpdir, neff_name)

    bu.compile_bir_kernel = patched
    b2j.compile_bir_kernel = patched
    _CACHE["shim"] = True


def build():
    _install_compiler_shim()
    import concourse.bass as bass  # noqa: F401
    import concourse.mybir as mybir
    import concourse.tile as tile
    from concourse import bacc

    fp32 = mybir.dt.float32
    bf16 = mybir.dt.bfloat16
    AF = mybir.ActivationFunctionType
    ALU = mybir.AluOpType

    nc = bacc.Bacc("TRN2", debug=False, target_bir_lowering=False, num_devices=NC)

    hsT = nc.declare_dram_parameter("hsT", [H, BS], bf16, isOutput=False)
    wqT = nc.declare_dram_parameter("wqT", [H, EL], bf16, isOutput=False)
    wkvT = nc.declare_dram_parameter("wkvT", [H, 2 * D], bf16, isOutput=False)
    wdT = nc.declare_dram_parameter("wdT", [H, EL], bf16, isOutput=False)
    cosq = nc.declare_dram_parameter("cosq", [128, BS], bf16, isOutput=False)
    sinq = nc.declare_dram_parameter("sinq", [128, BS], bf16, isOutput=False)
    trimask = nc.declare_dram_parameter("trimask", [128, 128], bf16, isOutput=False)
    ident = nc.declare_dram_parameter("ident", [64, 64], bf16, isOutput=False)
    outT = nc.declare_dram_parameter("outT", [EL, BS], fp32, isOutput=True)

    rg = [list(range(NC))]
    GPAIRS = ((0, 2, 4, 6), (1, 3, 5, 7))

    with tile.TileContext(nc, num_cores=NC) as tc:
        with (
            tc.tile_pool(name="const", bufs=1) as cp,
            tc.tile_pool(name="dram", bufs=1, space="DRAM") as dp,
        ):
            # ---- weight / const loads.  wkv first (kv proj gates startup),
            # then wq per-et slices; hs chunks go on the sync queue below.
            wkv_sb = cp.tile([128, HT, 2 * D], bf16)
            nc.gpsimd.dma_start(
                wkv_sb[:], wkvT.ap().rearrange("(a p) e -> p a e", p=128)
            )
            wq_sb = cp.tile([128, HT, EL], bf16)
            wq_r = wqT.ap().rearrange("(a p) e -> p a e", p=128)
            for et in range(4):
                nc.gpsimd.dma_start(
                    wq_sb[:, :, et * 128 : (et + 1) * 128],
                    wq_r[:, :, et * 128 : (et + 1) * 128],
                )
            cos_sb = cp.tile([128, BS], bf16)
            sin_sb = cp.tile([128, BS], bf16)
            nc.gpsimd.dma_start(cos_sb[:], cosq.ap())
            nc.gpsimd.dma_start(sin_sb[:], sinq.ap())
            tri_sb = cp.tile([128, 128], bf16)
            nc.gpsimd.dma_start(tri_sb[:], trimask.ap())
            id_sb = cp.tile([64, 64], bf16)
            nc.gpsimd.dma_start(id_sb[:], ident.ap())
            wd_sb = cp.tile([128, HT, EL], bf16)  # loaded later (super start)

            # ---- chunked collective buffers (one per batch x s-block)
            agin = [
                [dp.tile([EL, SBK], bf16, name=f"agin{b}_{sq}") for sq in range(NBLK)]
                for b in range(B)
            ]
            agout = [
                [
                    dp.tile(
                        [NC * EL, SBK],
                        bf16,
                        addr_space="Shared",
                        name=f"agout{b}_{sq}",
                    )
                    for sq in range(NBLK)
                ]
                for b in range(B)
            ]

            with (
                tc.tile_pool(name="mid", bufs=1) as mp,
                tc.tile_pool(name="scr", bufs=1) as sp,
            ):
                qT = [mp.tile([128, 4, S], bf16, name=f"qT{b}") for b in range(B)]
                kT = [mp.tile([128, S], bf16, name=f"kT{b}") for b in range(B)]
                v_ext = [
                    mp.tile([128, S // 128, D + 1], bf16, name=f"vx{b}")
                    for b in range(B)
                ]
                for b in range(B):
                    nc.gpsimd.memset(v_ext[b][:, :, D : D + 1], 1.0)

                def proj_units(b, pq, pvt, hs_ch):
                    """kv proj + k-RoPE + v-transpose, 'kv_done' marker, then
                    q proj + q-RoPE.  Yields at ~2us granule boundaries."""
                    bcol = slice(b * S, (b + 1) * S)
                    kvp = pq.tile([128, NBLK, SBK], fp32, tag="pp")
                    for ch in range(NCH):
                        hst = hs_ch[b][ch]
                        for a8 in range(CHA):
                            a = ch * CHA + a8
                            for blk in range(NBLK):
                                nc.tensor.matmul(
                                    kvp[:, blk, :],
                                    lhsT=wkv_sb[:, a, :],
                                    rhs=hst[:, a8, blk * SBK : (blk + 1) * SBK],
                                    start=(a == 0),
                                    stop=(a == HT - 1),
                                )
                        yield "kv"
                    kvraw = sp.tile([128, NBLK, SBK], bf16, tag="kvraw")
                    for blk in range(NBLK):
                        nc.scalar.copy(kvraw[:, blk, :], kvp[:, blk, :])
                    # k RoPE: rows 0:64 of kvraw hold k^T
                    ksh = sp.tile([64, NBLK, SBK], bf16, tag="ksh")
                    for half in range(2):
                        dst = slice(half * 32, half * 32 + 32)
                        src = slice((1 - half) * 32, (1 - half) * 32 + 32)
                        nc.scalar.dma_start(ksh[dst, :, :], kvraw[src, :, :])
                    kt2 = sp.tile([64, S], bf16, tag="kt2")
                    nc.vector.tensor_mul(
                        kT[b][0:64, :], kvraw[0:64, :, :], cos_sb[0:64, bcol]
                    )
                    nc.vector.tensor_mul(kt2[:], ksh[:], sin_sb[0:64, bcol])
                    nc.vector.tensor_add(kT[b][0:64, :], kT[b][0:64, :], kt2[:])
                    nc.scalar.dma_start(kT[b][64:128, :], kT[b][0:64, :])
                    yield "krope"
                    # v: transpose [d, t] -> [t, d] via PE
                    vtmp = sp.tile([64, NBLK, SBK], bf16, tag="vtmp")
                    for blk in range(NBLK):
                        nc.scalar.copy(vtmp[:, blk, :], kvraw[64:128, blk, :])
                    for blk in range(NBLK):
                        for j in range(SBK // 128):
                            vtp = pvt.tile([128, D], bf16, tag="vtp")
                            nc.tensor.transpose(
                                vtp[:],
                                vtmp[:, blk, j * 128 : (j + 1) * 128],
                                id_sb[:],
                            )
                            nc.scalar.copy(
                                v_ext[b][:, blk * (SBK // 128) + j, 0:D], vtp[:]
                            )
                    yield "kv_done"
                    # --- q projection + RoPE, one 128-wide e-tile at a time
                    for et in range(4):
                        qp = pq.tile([128, NBLK, SBK], fp32, tag="pp")
                        for ch in range(NCH):
                            hst = hs_ch[b][ch]
                            for a8 in range(CHA):
                                a = ch * CHA + a8
                                for blk in range(NBLK):
                                    nc.tensor.matmul(
                                        qp[:, blk, :],
                                        lhsT=wq_sb[:, a, et * 128 : (et + 1) * 128],
                                        rhs=hst[:, a8, blk * SBK : (blk + 1) * SBK],
                                        start=(a == 0),
                                        stop=(a == HT - 1),
                                    )
                            yield "q"
                        qraw = sp.tile([128, NBLK, SBK], bf16, tag="qraw")
                        for blk in range(NBLK):
                            nc.scalar.copy(qraw[:, blk, :], qp[:, blk, :])
                        qsh = sp.tile([128, NBLK, SBK], bf16, tag="qsh")
                        for hh in range(2):
                            for half in range(2):
                                dst = slice(
                                    hh * 64 + half * 32, hh * 64 + half * 32 + 32
                                )
                                src = slice(
                                    hh * 64 + (1 - half) * 32,
                                    hh * 64 + (1 - half) * 32 + 32,
                                )
                                nc.scalar.dma_start(qsh[dst, :, :], qraw[src, :, :])
                        t2 = sp.tile([128, S], bf16, tag="t2")
                        nc.vector.tensor_mul(
                            qT[b][:, et, :], qraw[:, :, :], cos_sb[:, bcol]
                        )
                        nc.vector.tensor_mul(t2[:], qsh[:, :, :], sin_sb[:, bcol])
                        nc.vector.tensor_add(qT[b][:, et, :], qT[b][:, et, :], t2[:])
                        yield "qrope"

                def attn_units(b, pst, ppv):
                    """Causal-trimmed attention for batch b; sq-outer so each
                    512-token block's AllGather fires when all heads done."""
                    for sq in range(NBLK):
                        for gi, gpair in enumerate(GPAIRS):
                            qrows = slice(gi * 64, gi * 64 + 64)
                            pv = ppv.tile(
                                [D + 1, 4, SBK], fp32, tag="pv", name=f"pv{b}{sq}{gi}"
                            )
                            ntile = 4 * sq + 4
                            for ti in range(ntile):
                                k = ti - 4 * sq
                                lo = max(k, 0) * 128
                                pts = {}
                                for g in gpair:
                                    stp = pst.tile([128, SBK], fp32, tag="st")
                                    nc.tensor.matmul(
                                        stp[:, lo:SBK],
                                        lhsT=kT[b][qrows, ti * 128 : (ti + 1) * 128],
                                        rhs=qT[b][
                                            qrows,
                                            g // 2,
                                            sq * SBK + lo : (sq + 1) * SBK,
                                        ],
                                        start=True,
                                        stop=True,
                                    )
                                    pT = sp.tile([128, SBK], bf16, tag="pt", bufs=6)
                                    nc.scalar.activation(
                                        pT[:, lo:SBK], stp[:, lo:SBK], AF.Exp,
                                        scale=INV,
                                    )
                                    if k >= 0:
                                        nc.vector.tensor_mul(
                                            pT[:, lo : lo + 128],
                                            pT[:, lo : lo + 128],
                                            tri_sb[:],
                                        )
                                    pts[g] = pT
                                yield "sc"
                                for gj, g in enumerate(gpair):
                                    nc.tensor.matmul(
                                        pv[:, gj, lo:SBK],
                                        lhsT=v_ext[b][:, ti, :],
                                        rhs=pts[g][:, lo:SBK],
                                        start=(ti == 0),
                                        stop=(ti == ntile - 1),
                                    )
                            # 1/rowsum scale + emit to the gather buffer
                            rc = sp.tile([1, 4, SBK], bf16, tag="rc", bufs=2)
                            with nc.allow_low_precision(reason="softmax recip bf16"):
                                for gj in range(4):
                                    nc.vector.reciprocal(
                                        rc[:, gj, :], pv[D : D + 1, gj, :]
                                    )
                            bcs = sp.tile([64, 4, SBK], bf16, tag="bcs", bufs=2)
                            nc.gpsimd.partition_broadcast(bcs[:], rc[:])
                            for gj, g in enumerate(gpair):
                                ao = sp.tile([64, SBK], bf16, tag="ao", bufs=3)
                                nc.vector.tensor_mul(
                                    ao[:], pv[0:D, gj, :], bcs[:, gj, :]
                                )
                                nc.sync.dma_start(
                                    agin[b][sq][g * 64 : (g + 1) * 64, :], ao[:]
                                )
                            yield "ao"
                        nc.gpsimd.collective_compute(
                            "AllGather",
                            ALU.bypass,
                            replica_groups=rg,
                            ins=[agin[b][sq][:].opt()],
                            outs=[agout[b][sq][:].opt()],
                        )

                def drive(gen, until=None):
                    for mark in gen:
                        if until is not None and mark == until:
                            return

                def interleave(g1, g2):
                    alive1 = alive2 = True
                    while alive1 or alive2:
                        if alive1:
                            alive1 = next(g1, None) is not None
                        if alive2:
                            alive2 = next(g2, None) is not None

                with tc.tile_pool(name="hsp", bufs=NCH) as hpp:
                    # all 16 hs chunk loads queued up front on the sync queue
                    hs_ch = [[None] * NCH for _ in range(B)]
                    hsT_r = hsT.ap().rearrange("(a p) s -> p a s", p=128)
                    for b in range(B):
                        for ch in range(NCH):
                            t = hpp.tile(
                                [128, CHA, S], bf16, tag="hs", name=f"hs{b}_{ch}"
                            )
                            nc.sync.dma_start(
                                t[:],
                                hsT_r[
                                    :,
                                    ch * CHA : (ch + 1) * CHA,
                                    b * S : (b + 1) * S,
                                ],
                            )
                            hs_ch[b][ch] = t

                    with tc.tile_pool(name="pq", bufs=1, space="PSUM") as pq:
                        with tc.tile_pool(
                            name="vt", bufs=2, space="PSUM"
                        ) as pvt:
                            p0 = proj_units(0, pq, pvt, hs_ch)
                            drive(p0)
                            nc.gpsimd.dma_start(
                                wd_sb[:],
                                wdT.ap().rearrange("(a p) e -> p a e", p=128),
                            )
                            p1 = proj_units(1, pq, pvt, hs_ch)
                            drive(p1, until="kv_done")
                        with (
                            tc.tile_pool(name="st", bufs=2, space="PSUM") as pst,
                            tc.tile_pool(name="pv", bufs=1, space="PSUM") as ppv,
                        ):
                            a0 = attn_units(0, pst, ppv)
                            interleave(p1, a0)
                            a1 = attn_units(1, pst, ppv)
                            drive(a1)

                # ---- dense: stream gathered activations in chunks
                with (
                    tc.tile_pool(name="dscr", bufs=1) as dsp,
                    tc.tile_pool(name="yp", bufs=1, space="PSUM") as pyp,
                ):
                    for b in range(B):
                        yp = pyp.tile([128, 8, SBK], fp32, tag="yp", name=f"yp{b}")
                        for ac in range(4):
                            agc = dsp.tile([128, 8, S], bf16, tag="agc", bufs=2)
                            for sq in range(NBLK):
                                nc.gpsimd.dma_start(
                                    agc[:, :, sq * SBK : (sq + 1) * SBK],
                                    agout[b][sq]
                                    .rearrange("(a p) s -> p a s", p=128)[
                                        :, ac * 8 : (ac + 1) * 8, :
                                    ],
                                )
                            for ot in range(4):
                                for a8 in range(8):
                                    for blk in range(NBLK):
                                        nc.tensor.matmul(
                                            yp[:, ot * NBLK + blk, :],
                                            lhsT=wd_sb[
                                                :,
                                                ac * 8 + a8,
                                                ot * 128 : (ot + 1) * 128,
                                            ],
                                            rhs=agc[
                                                :, a8, blk * SBK : (blk + 1) * SBK
                                            ],
                                            start=(ac == 0 and a8 == 0),
                                            stop=(ac == 3 and a8 == 7),
                                        )
                        for ot in range(4):
                            for blk in range(NBLK):
                                ysb = dsp.tile([128, SBK], fp32, tag="ysb", bufs=2)
                                nc.vector.tensor_copy(
                                    ysb[:], yp[:, ot * NBLK + blk, :]
                                )
                                col = b * S + blk * SBK
                                nc.sync.dma_start(
                                    outT.ap()[
                                        ot * 128 : (ot + 1) * 128, col : col + SBK
                                    ],
                                    ysb[:],
                                )

    nc.finalize()
    return nc


def _prep_inputs(hidden_states, cos, sin, wq, wk, wv, wd):
    bf = ml_dtypes.bfloat16
    hs2 = np.ascontiguousarray(hidden_states.reshape(BS, H).T.astype(bf))  # [H, BS]
    cosT = cos.T.astype(np.float32)  # [64, 1024]
    sinT = sin.T.astype(np.float32)
    sinS = np.concatenate([-sinT[0:32], sinT[32:64]], axis=0)
    cosq = np.ascontiguousarray(np.tile(cosT, (2, 2))).astype(bf)  # [128, 2048]
    sinq = np.ascontiguousarray(np.tile(sinS, (2, 2))).astype(bf)
    tri = np.triu(np.ones((128, 128), dtype=np.float32)).astype(bf)
    idn = np.eye(64, dtype=np.float32).astype(bf)
    in_maps = []
    for m in range(NC):
        wkv = np.concatenate(
            [wk[m * D : (m + 1) * D, :], wv[m * D : (m + 1) * D, :]], axis=0
        )  # [128, H]
        in_maps.append(
            {
                "hsT": hs2,
                "wqT": np.ascontiguousarray(wq[m * EL : (m + 1) * EL, :].T.astype(bf)),
                "wkvT": np.ascontiguousarray(wkv.T.astype(bf)),
                "wdT": np.ascontiguousarray(wd[m * EL : (m + 1) * EL, :].T.astype(bf)),
                "cosq": cosq,
                "sinq": sinq,
                "trimask": tri,
                "ident": idn,
            }
        )
    return in_maps


def kernel(hidden_states, alibi, attention_mask, cos, sin, wq, wk, wv, wd,
           _trace=False):
    from concourse.bass_utils import run_bass_kernel_spmd

    if "nc" not in _CACHE:
        _CACHE["nc"] = build()
    nc = _CACHE["nc"]
    in_maps = _prep_inputs(hidden_states, cos, sin, wq, wk, wv, wd)
    res = run_bass_kernel_spmd(nc, in_maps, core_ids=list(range(NC)), trace=_trace)
    _CACHE["last_result"] = res
    outs = [
        np.ascontiguousarray(res.results[m]["outT"].T).reshape(B, S, EL)
        for m in range(NC)
    ]
    return np.concatenate(outs, axis=-1).astype(np.float32)


# revision 25
# speedup vs baseline: 1.2651x; 1.2651x over previous
"""GQA attention forward (dense_transformer), 8-core tensor-parallel Bass kernel.

Problem (hardcoded): B=2, S=1024, H=4096, n_kv=8, G=8 (heads/kv), D=64, f32 io.
Sharding: core m owns kv-group m (8 q-heads + 1 kv-head), computes its slice
attnT_m = [512, 2048] of the attention output (feature-on-partition transposed
layout), AllGathers attnT (bf16, per batch, overlapped with compute), then
computes output columns y[:, m*512:(m+1)*512] = attn @ wd[m*512:(m+1)*512, :].T,
emitted transposed (yT); the host un-transposes and concatenates.

v3 schedule:
 - hs streamed in 4-htile chunks; proj(b0) runs kv+q0+q1 per chunk (3 PSUM
   groups) so the PE keeps pace with the DMA during startup.
 - attention(b0) interleaved at granule level with q-proj(b1) so the
   scores->exp->PV chain never idles the PE.
 - causal-trimmed scores/exp/PV streams (only s >= t columns).
 - V is extended with 64 ones-columns: PV emits the attn slice on PSUM
   rows 0:64 and the softmax row-sum replicated on rows 64:128, so the
   reciprocal runs on 64 DVE lanes (reciprocal_approx_fast) and one small
   partition-shift DMA replaces the (slow) gpsimd broadcast.
 - AllGather per batch: AG(b0) overlaps attention(b1); AG(b1) overlaps
   dense(b0); dense streams gathered activations in 4-htile chunks.

All matmuls bf16 (rel-err budget 2e-2); softmax skips max-subtraction
(logits bounded ~|7|); probs stay unnormalized through PV; output scaled by
1/sum. The BIR shim drops Ldweights whose stationary matches the previous
load, so same-stationary matmuls are emitted back to back.
"""

import sys

import numpy as np

for _p in ("/opt/trn_rl_repo",):
    if _p not in sys.path:
        sys.path.insert(0, _p)

import ml_dtypes

B, S, H = 2, 1024, 4096
NKV, G, D = 8, 8, 64
NC = 8
BS = B * S          # 2048 flattened tokens
EL = G * D          # 512 local attn features per core
HT = H // 128       # 32 h-tiles
SBK = 512           # s-block everywhere
NBLK = S // SBK     # 2 s-blocks per batch
NCH = 8             # hs chunks per batch (4 h-tiles each)
CHA = HT // NCH     # h-tiles per chunk = 4
INV = 0.125         # 1/sqrt(D)

_CACHE = {}


def _fix_bir_for_old_walrus(bir_json):
    """Adapt newer-concourse BIR to the container's older walrus:
    - register allocations need num_physical_regs set
    - only one sem-wait per instruction: hoist extras onto EventSemaphore nops
    - drop Ldweights identical to the previous one (stationary persists in
      the PE array; verified bit-exact on hardware)
    """
    import json

    bir = json.loads(bir_json)
    ndrop = 0
    for f in bir["functions"]:
        for a in f["allocations"]:
            if a.get("Skind") == "register" and not a.get("num_physical_regs"):
                a["num_physical_regs"] = 1
        for blk in f["blocks"]:
            newins = []
            last_ldw = None
            for ins in blk["instructions"]:
                si = ins.get("sync_info") or {}
                waits = si.get("on_wait") or []
                if len(waits) > 1:
                    for j, w in enumerate(waits[:-1]):
                        newins.append(
                            {
                                "engine": ins["engine"],
                                "ins": [],
                                "outs": [],
                                "name": f"{ins['name']}_w{j}",
                                "opcode": "EventSemaphore",
                                "sync_info": {"on_update": [], "on_wait": [w]},
                                "debug": ins.get("debug"),
                            }
                        )
                    si["on_wait"] = [waits[-1]]
                op = ins.get("opcode")
                if op == "Ldweights":
                    key = json.dumps(ins["ins"], sort_keys=True)
                    sync = ins.get("sync_info") or {}
                    if (
                        key == last_ldw
                        and not sync.get("on_wait")
                        and not sync.get("on_update")
                    ):
                        ndrop += 1
                        continue
                    last_ldw = key
                elif op != "Matmult":
                    last_ldw = None
                newins.append(ins)
            blk["instructions"] = newins
    return json.dumps(bir).encode()


def _install_compiler_shim():
    if _CACHE.get("shim"):
        return
    import concourse.bass_utils as bu
    import concourse.bass2jax as b2j

    orig = getattr(bu.compile_bir_kernel, "__wrapped__", bu.compile_bir_kernel)

    def patched(bir_json, tmpdir, neff_name="file.neff"):
        return orig(_fix_bir_for_old_walrus(bir_json), tmpdir, neff_name)

    bu.compile_bir_kernel = patched
    b2j.compile_bir_kernel = patched
    _CACHE["shim"] = True


def build():
    _install_compiler_shim()
    import concourse.bass as bass  # noqa: F401
    import concourse.mybir as mybir
    import concourse.tile as tile
    from concourse import bacc

    fp32 = mybir.dt.float32
    bf16 = mybir.dt.bfloat16
    AF = mybir.ActivationFunctionType
    ALU = mybir.AluOpType

    nc = bacc.Bacc("TRN2", debug=False, target_bir_lowering=False, num_devices=NC)

    hsT = nc.declare_dram_parameter("hsT", [H, BS], bf16, isOutput=False)
    wqT = nc.declare_dram_parameter("wqT", [H, EL], bf16, isOutput=False)
    wkvT = nc.declare_dram_parameter("wkvT", [H, 2 * D], bf16, isOutput=False)
    wdT = nc.declare_dram_parameter("wdT", [H, EL], bf16, isOutput=False)
    cosq = nc.declare_dram_parameter("cosq", [128, S], bf16, isOutput=False)
    sinq = nc.declare_dram_parameter("sinq", [128, S], bf16, isOutput=False)
    trimask = nc.declare_dram_parameter("trimask", [128, 128], bf16, isOutput=False)
    ident = nc.declare_dram_parameter("ident", [64, 64], bf16, isOutput=False)
    outT = nc.declare_dram_parameter("outT", [EL, BS], fp32, isOutput=True)

    rg = [list(range(NC))]
    GPAIRS = ((0, 2, 4, 6), (1, 3, 5, 7))

    with tile.TileContext(nc, num_cores=NC) as tc:
        with (
            tc.tile_pool(name="const", bufs=1) as cp,
            tc.tile_pool(name="dram", bufs=1, space="DRAM") as dp,
        ):
            # ---- weight / const loads (gpsimd queue; hs on sync queue below)
            wkv_sb = cp.tile([128, HT, 2 * D], bf16)
            nc.gpsimd.dma_start(
                wkv_sb[:], wkvT.ap().rearrange("(a p) e -> p a e", p=128)
            )
            cos_sb = cp.tile([128, S], bf16)
            sin_sb = cp.tile([128, S], bf16)
            nc.gpsimd.dma_start(cos_sb[:], cosq.ap())
            nc.gpsimd.dma_start(sin_sb[:], sinq.ap())
            tri_sb = cp.tile([128, 128], bf16)
            nc.gpsimd.dma_start(tri_sb[:], trimask.ap())
            id_sb = cp.tile([64, 64], bf16)
            nc.gpsimd.dma_start(id_sb[:], ident.ap())
            ones_sb = cp.tile([1, 64], bf16)
            nc.gpsimd.memset(ones_sb[:], 1.0)
            wq_sb = cp.tile([128, HT, EL], bf16)
            wq_r = wqT.ap().rearrange("(a p) e -> p a e", p=128)
            for et in range(4):
                nc.gpsimd.dma_start(
                    wq_sb[:, :, et * 128 : (et + 1) * 128],
                    wq_r[:, :, et * 128 : (et + 1) * 128],
                )
            wd_sb = cp.tile([128, HT, EL], bf16)  # loaded at super start

            agin = [dp.tile([EL, S], bf16, name=f"agin{b}") for b in range(B)]
            agout = [
                dp.tile([NC * EL, S], bf16, addr_space="Shared", name=f"agout{b}")
                for b in range(B)
            ]

            with (
                tc.tile_pool(name="mid", bufs=1) as mp,
                tc.tile_pool(name="scr", bufs=1) as sp,
            ):
                qT = [mp.tile([128, 4, S], bf16, name=f"qT{b}") for b in range(B)]
                kT = [mp.tile([128, S], bf16, name=f"kT{b}") for b in range(B)]
                # v extended with a ones-column: PV row 64 = softmax row-sum
                v_ext = [
                    mp.tile([128, S // 128, D + 1], bf16, name=f"vx{b}")
                    for b in range(B)
                ]
                for b in range(B):
                    nc.gpsimd.memset(v_ext[b][:, :, D : D + 1], 1.0)

                pools = {}

                def proj_units(b, grouped):
                    """kv proj (+ q et0/et1 when grouped) chunk-paced, then
                    k-RoPE + v-transpose ('kv_done'), then remaining q e-tiles
                    + RoPE.  Yields at ~2us granule boundaries."""
                    bcol = slice(b * S, (b + 1) * S)
                    pqA = pools["pqA"]
                    kvp = pqA.tile([128, NBLK, SBK], fp32, tag="kvp")
                    qps = {}
                    first = (0, 1) if grouped else ()
                    for et in first:
                        qps[et] = pqA.tile(
                            [128, NBLK, SBK], fp32, tag=f"qp{et}", name=f"qp{b}_{et}"
                        )
                    hs_ch = pools["hs"][b]
                    for ch in range(NCH):
                        hst = hs_ch[ch]
                        for a8 in range(CHA):
                            a = ch * CHA + a8
                            for blk in range(NBLK):
                                nc.tensor.matmul(
                                    kvp[:, blk, :],
                                    lhsT=wkv_sb[:, a, :],
                                    rhs=hst[:, a8, blk * SBK : (blk + 1) * SBK],
                                    start=(a == 0),
                                    stop=(a == HT - 1),
                                )
                        for et in first:
                            for a8 in range(CHA):
                                a = ch * CHA + a8
                                for blk in range(NBLK):
                                    nc.tensor.matmul(
                                        qps[et][:, blk, :],
                                        lhsT=wq_sb[:, a, et * 128 : (et + 1) * 128],
                                        rhs=hst[:, a8, blk * SBK : (blk + 1) * SBK],
                                        start=(a == 0),
                                        stop=(a == HT - 1),
                                    )
                        yield "kv"
                    kvraw = sp.tile([128, NBLK, SBK], bf16, tag="kvraw")
                    for blk in range(NBLK):
                        nc.scalar.copy(kvraw[:, blk, :], kvp[:, blk, :])
                    # k RoPE: rows 0:64 of kvraw hold k^T
                    ksh = sp.tile([64, NBLK, SBK], bf16, tag="ksh")
                    for half in range(2):
                        dst = slice(half * 32, half * 32 + 32)
                        src = slice((1 - half) * 32, (1 - half) * 32 + 32)
                        nc.scalar.dma_start(ksh[dst, :, :], kvraw[src, :, :])
                    kt2 = sp.tile([64, S], bf16, tag="kt2")
                    nc.vector.tensor_mul(
                        kT[b][0:64, :], kvraw[0:64, :, :], cos_sb[0:64, :]
                    )
                    nc.vector.tensor_mul(kt2[:], ksh[:], sin_sb[0:64, :])
                    nc.vector.tensor_add(kT[b][0:64, :], kT[b][0:64, :], kt2[:])
                    nc.scalar.dma_start(kT[b][64:128, :], kT[b][0:64, :])
                    yield "krope"
                    # v: transpose [d, t] -> [t, d] via PE
                    vtmp = sp.tile([64, NBLK, SBK], bf16, tag="vtmp")
                    for blk in range(NBLK):
                        nc.scalar.copy(vtmp[:, blk, :], kvraw[64:128, blk, :])
                    for blk in range(NBLK):
                        for j in range(SBK // 128):
                            vtp = pools["vt"].tile([128, D], bf16, tag="vtp")
                            nc.tensor.transpose(
                                vtp[:],
                                vtmp[:, blk, j * 128 : (j + 1) * 128],
                                id_sb[:],
                            )
                            nc.scalar.copy(
                                v_ext[b][:, blk * (SBK // 128) + j, 0:D], vtp[:]
                            )
                    yield "kv_done"

                    def qrope(et, qp):
                        qraw = sp.tile([128, NBLK, SBK], bf16, tag="qraw")
                        for blk in range(NBLK):
                            nc.vector.tensor_copy(qraw[:, blk, :], qp[:, blk, :])
                        qsh = sp.tile([128, NBLK, SBK], bf16, tag="qsh")
                        for hh in range(2):
                            for half in range(2):
                                dst = slice(
                                    hh * 64 + half * 32, hh * 64 + half * 32 + 32
                                )
                                src = slice(
                                    hh * 64 + (1 - half) * 32,
                                    hh * 64 + (1 - half) * 32 + 32,
                                )
                                nc.scalar.dma_start(qsh[dst, :, :], qraw[src, :, :])
                        t2 = sp.tile([128, S], bf16, tag="t2")
                        nc.vector.tensor_mul(
                            qT[b][:, et, :], qraw[:, :, :], cos_sb[:, :]
                        )
                        nc.vector.tensor_mul(t2[:], qsh[:, :, :], sin_sb[:, :])
                        nc.vector.tensor_add(qT[b][:, et, :], qT[b][:, et, :], t2[:])

                    for et in first:
                        qrope(et, qps[et])
                        yield "qrope"
                    rest = (2, 3) if grouped else (0, 1, 2, 3)
                    # grouped (b0): reuse pqA tag space; else (b1): pqB
                    for i, et in enumerate(rest):
                        if grouped:
                            qp = pqA.tile(
                                [128, NBLK, SBK], fp32, tag=("kvp", "qp0")[i],
                                name=f"qpr{b}_{et}",
                            )
                        else:
                            qp = pools["pqB"].tile(
                                [128, NBLK, SBK], fp32, tag="pp", name=f"qpr{b}_{et}"
                            )
                        for ch in range(NCH):
                            hst = hs_ch[ch]
                            for a8 in range(CHA):
                                a = ch * CHA + a8
                                for blk in range(NBLK):
                                    nc.tensor.matmul(
                                        qp[:, blk, :],
                                        lhsT=wq_sb[:, a, et * 128 : (et + 1) * 128],
                                        rhs=hst[:, a8, blk * SBK : (blk + 1) * SBK],
                                        start=(a == 0),
                                        stop=(a == HT - 1),
                                    )
                            yield "q"
                        qrope(et, qp)
                        yield "qrope"

                def attn_units(b):
                    """Causal-trimmed attention for batch b; AllGather for the
                    batch fires as soon as the last head's slice lands.
                    The 1/sum scale is pipelined one gpair late: the PE
                    broadcasts exp(-ln(sum)) into the freed pv bank with a
                    K=1 ones-matmul, so no gpsimd op sits in the chain."""
                    pst, ppv = pools["pst"], pools["ppv"]
                    pending = [None]

                    def finish():
                        if pending[0] is None:
                            return
                        pv_, pvs_, rcp_, gpair_, sq_ = pending[0]
                        pending[0] = None
                        for gj in range(4):
                            nc.tensor.matmul(
                                pv_[0:D, gj, :],
                                lhsT=ones_sb[:],
                                rhs=rcp_[:, gj, :],
                                start=True,
                                stop=True,
                            )
                        for gj, g in enumerate(gpair_):
                            ao = sp.tile([64, SBK], bf16, tag="ao", bufs=3)
                            nc.vector.tensor_mul(
                                ao[:], pvs_[0:D, gj, :], pv_[0:D, gj, :]
                            )
                            nc.sync.dma_start(
                                agin[b][
                                    g * 64 : (g + 1) * 64,
                                    sq_ * SBK : (sq_ + 1) * SBK,
                                ],
                                ao[:],
                            )

                    for sq in range(NBLK):
                        for gi, gpair in enumerate(GPAIRS):
                            qrows = slice(gi * 64, gi * 64 + 64)
                            pv = ppv.tile(
                                [D + 1, 4, SBK], fp32, tag="pv", name=f"pv{b}{sq}{gi}"
                            )
                            ntile = 4 * sq + 4
                            for ti in range(ntile):
                                k = ti - 4 * sq
                                lo = max(k, 0) * 128
                                pts = {}
                                for g in gpair:
                                    stp = pst.tile([128, SBK], fp32, tag="st")
                                    nc.tensor.matmul(
                                        stp[:, lo:SBK],
                                        lhsT=kT[b][qrows, ti * 128 : (ti + 1) * 128],
                                        rhs=qT[b][
                                            qrows,
                                            g // 2,
                                            sq * SBK + lo : (sq + 1) * SBK,
                                        ],
                                        start=True,
                                        stop=True,
                                    )
                                    pT = sp.tile([128, SBK], bf16, tag="pt", bufs=6)
                                    nc.scalar.activation(
                                        pT[:, lo:SBK], stp[:, lo:SBK], AF.Exp,
                                        scale=INV,
                                    )
                                    if k >= 0:
                                        nc.vector.tensor_mul(
                                            pT[:, lo : lo + 128],
                                            pT[:, lo : lo + 128],
                                            tri_sb[:],
                                        )
                                    pts[g] = pT
                                yield "sc"
                                if ti == 0:
                                    finish()
                                for gj, g in enumerate(gpair):
                                    nc.tensor.matmul(
                                        pv[:, gj, lo:SBK],
                                        lhsT=v_ext[b][:, ti, :],
                                        rhs=pts[g][:, lo:SBK],
                                        start=(ti == 0),
                                        stop=(ti == ntile - 1),
                                    )
                            # free the PSUM bank with a copy, then compute
                            # 1/sum = exp(-ln(sum)) on scalar; the PE bcast +
                            # scale run at the start of the NEXT gpair
                            pvs = sp.tile([D + 1, 4, SBK], bf16, tag="pvs", bufs=1)
                            nc.vector.tensor_copy(pvs[:], pv[:, :, :])
                            rcl = sp.tile([1, 4, SBK], fp32, tag="rcl", bufs=1)
                            nc.scalar.activation(
                                rcl[:], pvs[D : D + 1, :, :], AF.Ln
                            )
                            rcp = sp.tile([1, 4, SBK], bf16, tag="rcp", bufs=1)
                            nc.scalar.activation(
                                rcp[:], rcl[:], AF.Exp, scale=-1.0
                            )
                            pending[0] = (pv, pvs, rcp, gpair, sq)
                            yield "ao"
                    finish()
                    nc.gpsimd.collective_compute(
                        "AllGather",
                        ALU.bypass,
                        replica_groups=rg,
                        ins=[agin[b][:].opt()],
                        outs=[agout[b][:].opt()],
                    )

                def drive(gen, until=None):
                    for mark in gen:
                        if until is not None and mark == until:
                            return

                def interleave(g1, g2):
                    # g2 (attention) driven 3:1 ahead of g1 (projection) so
                    # its AllGather fires as early as possible; the proj
                    # granule between attention units still hides exp latency
                    alive1 = alive2 = True
                    while alive1 or alive2:
                        for _ in range(3):
                            if alive2:
                                alive2 = next(g2, None) is not None
                        if alive1:
                            alive1 = next(g1, None) is not None

                with tc.tile_pool(name="hsp", bufs=NCH) as hpp:
                    # all 16 hs chunk loads queued up front on the sync queue
                    hs_ch = [[None] * NCH for _ in range(B)]
                    hsT_r = hsT.ap().rearrange("(a p) s -> p a s", p=128)
                    for b in range(B):
                        for ch in range(NCH):
                            t = hpp.tile(
                                [128, CHA, S], bf16, tag="hs", name=f"hs{b}_{ch}"
                            )
                            nc.sync.dma_start(
                                t[:],
                                hsT_r[
                                    :, ch * CHA : (ch + 1) * CHA, b * S : (b + 1) * S
                                ],
                            )
                            hs_ch[b][ch] = t
                    pools["hs"] = hs_ch

                    with tc.tile_pool(name="pqA", bufs=1, space="PSUM") as pqA:
                        pools["pqA"] = pqA
                        with tc.tile_pool(name="vt", bufs=2, space="PSUM") as pvt:
                            pools["vt"] = pvt
                            p0 = proj_units(0, grouped=True)
                            drive(p0)
                            nc.gpsimd.dma_start(
                                wd_sb[:],
                                wdT.ap().rearrange("(a p) e -> p a e", p=128),
                            )
                            p1 = proj_units(1, grouped=False)
                            drive(p1, until="kv_done")
                    with (
                        tc.tile_pool(name="pqB", bufs=1, space="PSUM") as pqB,
                        tc.tile_pool(name="st", bufs=2, space="PSUM") as pst,
                        tc.tile_pool(name="pv", bufs=1, space="PSUM") as ppv,
                    ):
                        pools["pqB"] = pqB
                        pools["pst"] = pst
                        pools["ppv"] = ppv
                        a0 = attn_units(0)
                        interleave(p1, a0)
                        a1 = attn_units(1)
                        drive(a1)

                # ---- dense: stream gathered activations in 4-htile chunks
                with (
                    tc.tile_pool(name="dscr", bufs=1) as dsp,
                    tc.tile_pool(name="yp", bufs=1, space="PSUM") as pyp,
                ):
                    for b in range(B):
                        yp = pyp.tile([128, 8, SBK], fp32, tag="yp", name=f"yp{b}")
                        agr = agout[b].rearrange("(a p) s -> p a s", p=128)
                        for ac in range(4):
                            agc = dsp.tile([128, 8, S], bf16, tag="agc", bufs=2)
                            nc.gpsimd.dma_start(
                                agc[:], agr[:, ac * 8 : (ac + 1) * 8, :]
                            )
                            for ot in range(4):
                                for a8 in range(8):
                                    for blk in range(NBLK):
                                        nc.tensor.matmul(
                                            yp[:, ot * NBLK + blk, :],
                                            lhsT=wd_sb[
                                                :,
                                                ac * 8 + a8,
                                                ot * 128 : (ot + 1) * 128,
                                            ],
                                            rhs=agc[
                                                :, a8, blk * SBK : (blk + 1) * SBK
                                            ],
                                            start=(ac == 0 and a8 == 0),
                                            stop=(ac == 3 and a8 == 7),
                                        )
                        for ot in range(4):
                            for blk in range(NBLK):
                                ysb = dsp.tile([128, SBK], fp32, tag="ysb", bufs=2)
                                nc.vector.tensor_copy(
                                    ysb[:], yp[:, ot * NBLK + blk, :]
                                )
                                col = b * S + blk * SBK
                                nc.sync.dma_start(
                                    outT.ap()[
                                        ot * 128 : (ot + 1) * 128, col : col + SBK
                                    ],
                                    ysb[:],
                                )

    nc.finalize()
    return nc


def _prep_inputs(hidden_states, cos, sin, wq, wk, wv, wd):
    bf = ml_dtypes.bfloat16
    hs2 = np.ascontiguousarray(hidden_states.reshape(BS, H).T.astype(bf))  # [H, BS]
    cosT = cos.T.astype(np.float32)  # [64, 1024]
    sinT = sin.T.astype(np.float32)
    sinS = np.concatenate([-sinT[0:32], sinT[32:64]], axis=0)
    cosq = np.ascontiguousarray(np.tile(cosT, (2, 1))).astype(bf)  # [128, 1024]
    sinq = np.ascontiguousarray(np.tile(sinS, (2, 1))).astype(bf)
    tri = np.triu(np.ones((128, 128), dtype=np.float32)).astype(bf)
    idn = np.eye(64, dtype=np.float32).astype(bf)
    in_maps = []
    for m in range(NC):
        wkv = np.concatenate(
            [wk[m * D : (m + 1) * D, :], wv[m * D : (m + 1) * D, :]], axis=0
        )  # [128, H]
        in_maps.append(
            {
                "hsT": hs2,
                "wqT": np.ascontiguousarray(wq[m * EL : (m + 1) * EL, :].T.astype(bf)),
                "wkvT": np.ascontiguousarray(wkv.T.astype(bf)),
                "wdT": np.ascontiguousarray(wd[m * EL : (m + 1) * EL, :].T.astype(bf)),
                "cosq": cosq,
                "sinq": sinq,
                "trimask": tri,
                "ident": idn,
            }
        )
    return in_maps


def kernel(hidden_states, alibi, attention_mask, cos, sin, wq, wk, wv, wd,
           _trace=False):
    from concourse.bass_utils import run_bass_kernel_spmd

    if "nc" not in _CACHE:
        _CACHE["nc"] = build()
    nc = _CACHE["nc"]
    in_maps = _prep_inputs(hidden_states, cos, sin, wq, wk, wv, wd)
    res = run_bass_kernel_spmd(nc, in_maps, core_ids=list(range(NC)), trace=_trace)
    _CACHE["last_result"] = res
    outs = [
        np.ascontiguousarray(res.results[m]["outT"].T).reshape(B, S, EL)
        for m in range(NC)
    ]
    return np.concatenate(outs, axis=-1).astype(np.float32)
